# revision 11
# baseline (speedup 1.0000x reference)
# Multi-head attention (B=4, L=2048, E=256, H=8) on 8 TRN2 NeuronCores.
#
# Sharding: core c handles batch b = c//2 and head group g = c%2 (heads
# 4g..4g+3).  Each core computes the partial output
#   sum_{h in group} softmax(x M_h x^T) (x N_h)
# for its batch, with host-folded per-head weights:
#   M_h = Wq_h Wk_h^T / sqrt(E)   (so scores = x M_h x^T)
#   N_h = Wv_h Wout_h             (so attn @ v @ Wout_h = (attn @ x) N_h)
# The host adds the two head-group partials per batch.
#
# Precision strategy (rel-err budget is 2e-2; this lands ~5e-3):
#   - scores matmuls (uT = M^T x^T and sT = xT^T uT) run in fp8 e4m3 with
#     perf_mode=DoubleRow: contraction of 256 packed as [128, 2, N] k-tile
#     pairs, one PE pass instead of two (~1.8x on the scores GEMMs).
#     M is pre-scaled by 1024 on the host so u-values sit in e4m3's
#     normal range; exp() unscales via its free `scale` operand.
#   - p = exp(s), wT = x^T p, and the out-projection run in bf16
#     (bf16 matmul = fp32r rate, but enables FWL weight loads and 1024-wide
#     moving operands).
#   - PSUM accumulation is fp32 throughout; softmax normalization (row
#     scaling) is applied after the output projection where it commutes.
#
# Per-core dataflow, per (head, 1024-wide qi block):
#   for each of 16 kj tiles: sT = DR-matmul(xT8_kj, uT8_qb)  (PSUM [128,1024])
#     pt = exp(sT/1024) -> bf16 SBUF (ACT, one 1024-wide op per tile)
#     w_psA += xn_kj^T pt[:, 0:512]  (both e-halves into one 2-bank tile)
#     bf16 pair-tree on DVE accumulates colacc = sum_kj pt
#   wTA = cast(w_psA); second pass over pt for qi 512:1024 -> wTB
#   rowsum via 8 tiny matmuls (colacc chunks ^T @ ones), one reciprocal
#   pj = wT^T N_h (PSUM), out_acc += pj * recip  (DVE scalar_tensor_tensor)
# Scores never touch HBM.

import numpy as np

B, L, E, H = 4, 2048, 256, 8
HL = H // 2          # heads per core
QB = 1024            # qi block width
NQB = L // QB        # 2
KT = L // 128        # 16 kj tiles
HE = H * E

_cache = {}


def _build_nc():
    import concourse.mybir as mybir
    from concourse import bacc
    from concourse.tile import TileContext

    F32 = mybir.dt.float32
    BF16 = mybir.dt.bfloat16
    F8 = mybir.dt.float8e4
    Exp = mybir.ActivationFunctionType.Exp
    DR = mybir.MatmulPerfMode.DoubleRow

    nc = bacc.Bacc(None, target_bir_lowering=False)

    x_d = nc.dram_tensor("x", [L, E], BF16, kind="ExternalInput")
    xt8_d = nc.dram_tensor("xt8", [128, 2, L], F8, kind="ExternalInput")
    xtb_d = nc.dram_tensor("xtb", [128, 2, L], BF16, kind="ExternalInput")
    mb_d = nc.dram_tensor("mb", [128, 2, HL * E], BF16, kind="ExternalInput")
    n_d = nc.dram_tensor("n", [128, 2, HL * E], BF16, kind="ExternalInput")
    out_d = nc.dram_tensor("out", [L, E], F32, kind="ExternalOutput")

    with TileContext(nc) as tc:
        with (
            tc.tile_pool(name="const", bufs=1) as cpool,
            tc.tile_pool(name="head", bufs=2) as hpool,
            tc.tile_pool(name="pt", bufs=18) as ptpool,
            tc.tile_pool(name="l1", bufs=5) as l1pool,
            tc.tile_pool(name="l2", bufs=3) as l2pool,
            tc.tile_pool(name="l3", bufs=2) as l3pool,
            tc.tile_pool(name="cacc", bufs=2) as capool,
            tc.tile_pool(name="wt", bufs=3) as wtpool,
            tc.tile_pool(name="rc", bufs=2) as rcpool,
            tc.tile_pool(name="ps_s", bufs=2, space="PSUM") as ps_s,
            tc.tile_pool(name="ps_w", bufs=1, space="PSUM") as ps_w,
            tc.tile_pool(name="ps_pj", bufs=2, space="PSUM") as ps_pj,
        ):
            ones = cpool.tile([128, 1], BF16, name="ones")
            nc.vector.memset(ones, 1.0)

            # ---- resident inputs ----
            mb = cpool.tile([128, 2, HL * E], BF16, name="mb")
            nc.sync.dma_start(mb, mb_d[:, :, :])
            xtb = cpool.tile([128, 2, L], BF16, name="xtb")
            for nb in range(NQB):
                nc.sync.dma_start(xtb[:, :, nb * QB:(nb + 1) * QB],
                                  xtb_d[:, :, nb * QB:(nb + 1) * QB])
            xt8 = cpool.tile([128, 2, L], F8, name="xt8")
            for nb in range(NQB):
                nc.sync.dma_start(xt8[:, :, nb * QB:(nb + 1) * QB],
                                  xt8_d[:, :, nb * QB:(nb + 1) * QB])
            xn = [cpool.tile([128, E], BF16, name=f"xn{t}") for t in range(KT)]
            for t in range(KT):
                nc.sync.dma_start(xn[t], x_d[t * 128:(t + 1) * 128, :])
            nsb = cpool.tile([128, 2, HL * E], BF16, name="nsb")
            nc.sync.dma_start(nsb, n_d[:, :, :])

            out_acc = [cpool.tile([128, E], F32, name=f"oacc{t}")
                       for t in range(KT)]

            # uT8 = (1024 * M_h)^T x^T in bf16, cast to fp8-packed [128, 2, L]
            uT8s = {}

            def u_phase(h):
                uT8 = hpool.tile([128, 2, L], F8, name="uT8", tag="uT8")
                uT8s[h] = uT8
                for eh in range(2):
                    for nb in range(NQB):
                        u_ps = ps_s.tile([128, QB], F32, name="ups", tag="s")
                        for sh in range(2):
                            for ih in range(2):
                                nc.tensor.matmul(
                                    u_ps[:, sh * 512:(sh + 1) * 512],
                                    mb[:, ih,
                                       h * E + eh * 128:h * E + (eh + 1) * 128],
                                    xtb[:, ih,
                                        nb * QB + sh * 512:
                                        nb * QB + (sh + 1) * 512],
                                    start=(ih == 0), stop=(ih == 1),
                                )
                        nc.vector.tensor_copy(
                            uT8[:, eh, nb * QB:(nb + 1) * QB], u_ps)

            u_phase(0)
            for h in range(HL):
                uT8 = uT8s[h]
                for qb in range(NQB):
                    if qb == NQB - 1 and h + 1 < HL:
                        u_phase(h + 1)  # prefetch next head's uT8
                    pts = []
                    l1 = []
                    w_ps = ps_w.tile([128, QB], F32, name="wpsA", tag="w")
                    for t in range(KT):
                        s_ps = ps_s.tile([128, QB], F32, name="sps", tag="s")
                        for sh in range(2):
                            nc.tensor.matmul(
                                s_ps[:, sh * 512:(sh + 1) * 512],
                                xt8[:, :, t * 128:(t + 1) * 128],
                                uT8[:, :,
                                    qb * QB + sh * 512:qb * QB + (sh + 1) * 512],
                                start=True, stop=True, perf_mode=DR,
                            )
                        pt = ptpool.tile([128, QB], BF16, name="pt", tag="pt")
                        nc.scalar.activation(pt, s_ps, Exp, scale=1.0 / 1024.0)
                        pts.append(pt)
                        for eh in range(2):
                            nc.tensor.matmul(
                                w_ps[:, eh * 512:(eh + 1) * 512],
                                xn[t][:, eh * 128:(eh + 1) * 128],
                                pt[:, 0:512],
                                start=(t == 0), stop=(t == KT - 1),
                            )
                        # bf16 pair tree, level 1 (defer the last pair so the
                        # wTA cast reaches the DVE queue first)
                        if t % 2 == 1 and t < KT - 1:
                            s1 = l1pool.tile([128, QB], BF16, name="s1",
                                             tag="l1")
                            nc.vector.tensor_add(s1, pts[t - 1], pts[t])
                            l1.append(s1)
                    wTA = wtpool.tile([128, QB], BF16, name="wTA", tag="wt")
                    nc.vector.tensor_copy(wTA, w_ps)
                    # finish the reduction tree
                    s1 = l1pool.tile([128, QB], BF16, name="s1", tag="l1")
                    nc.vector.tensor_add(s1, pts[KT - 2], pts[KT - 1])
                    l1.append(s1)
                    l2 = []
                    for i in range(4):
                        s2 = l2pool.tile([128, QB], BF16, name="s2", tag="l2")
                        nc.vector.tensor_add(s2, l1[2 * i], l1[2 * i + 1])
                        l2.append(s2)
                    l3 = []
                    for i in range(2):
                        s3 = l3pool.tile([128, QB], BF16, name="s3", tag="l3")
                        nc.vector.tensor_add(s3, l2[2 * i], l2[2 * i + 1])
                        l3.append(s3)
                    colacc = capool.tile([128, QB], BF16, name="colacc",
                                         tag="cacc")
                    nc.vector.tensor_add(colacc, l3[0], l3[1])

                    # second qi-half pass over the same pt tiles
                    w_ps = ps_w.tile([128, QB], F32, name="wpsB", tag="w")
                    for t in range(KT):
                        for eh in range(2):
                            nc.tensor.matmul(
                                w_ps[:, eh * 512:(eh + 1) * 512],
                                xn[t][:, eh * 128:(eh + 1) * 128],
                                pts[t][:, 512:1024],
                                start=(t == 0), stop=(t == KT - 1),
                            )
                    wTB = wtpool.tile([128, QB], BF16, name="wTB", tag="wt")
                    nc.vector.tensor_copy(wTB, w_ps)

                    # softmax denominators for the 8 qi chunks of this block
                    rs = ps_pj.tile([128, 8], F32, name="rs", tag="pj")
                    for j in range(8):
                        nc.tensor.matmul(rs[:, j:j + 1],
                                         colacc[:, j * 128:(j + 1) * 128],
                                         ones, start=True, stop=True)
                    recip = rcpool.tile([128, 8], F32, name="recip", tag="rc")
                    nc.vector.reciprocal(recip, rs)

                    for j in range(8):
                        wT = wTA if j < 4 else wTB
                        jj = j % 4
                        pj = ps_pj.tile([128, E], F32, name="pj", tag="pj")
                        for eh in range(2):
                            nc.tensor.matmul(
                                pj,
                                wT[:, eh * 512 + jj * 128:
                                   eh * 512 + (jj + 1) * 128],
                                nsb[:, eh, h * E:(h + 1) * E],
                                start=(eh == 0), stop=(eh == 1),
                            )
                        gt = qb * 8 + j
                        if h == 0:
                            nc.vector.tensor_scalar_mul(
                                out_acc[gt], pj, recip[:, j:j + 1])
                        else:
                            nc.vector.scalar_tensor_tensor(
                                out_acc[gt], pj, recip[:, j:j + 1],
                                out_acc[gt],
                                op0=mybir.AluOpType.mult,
                                op1=mybir.AluOpType.add)
                        if h == HL - 1:
                            nc.sync.dma_start(
                                out_d[gt * 128:(gt + 1) * 128, :],
                                out_acc[gt])

    nc.compile()
    return nc


def _get_nc():
    if "nc" not in _cache:
        _cache["nc"] = _build_nc()
    return _cache["nc"]


def _in_maps(x, W_qkv, W_out):
    import ml_dtypes

    f8 = ml_dtypes.float8_e4m3
    bf16 = ml_dtypes.bfloat16

    x = np.ascontiguousarray(np.asarray(x, dtype=np.float32))
    W_qkv = np.asarray(W_qkv, dtype=np.float32)
    W_out = np.asarray(W_out, dtype=np.float32)

    # Host-side weight folding (float64 for exactness):
    #   M_h = Wq_h Wk_h^T / sqrt(E) * 1024  (fp8 range lift),  N_h = Wv_h Wout_h
    Wq = W_qkv[:, 0:HE].astype(np.float64)
    Wk = W_qkv[:, HE:2 * HE].astype(np.float64)
    Wv = W_qkv[:, 2 * HE:3 * HE].astype(np.float64)
    Wo = W_out.astype(np.float64)
    scale = 1024.0 / np.sqrt(E)
    M = np.empty((H, E, E), np.float64)
    N = np.empty((H, E, E), np.float64)
    for h in range(H):
        M[h] = (Wq[:, h * E:(h + 1) * E] @ Wk[:, h * E:(h + 1) * E].T) * scale
        N[h] = Wv[:, h * E:(h + 1) * E] @ Wo[h * E:(h + 1) * E, :]

    maps = []
    for c in range(2 * B):
        b, g = c // 2, c % 2
        hs = HL * g
        mcat = np.concatenate([M[hs + i] for i in range(HL)], axis=1)
        ncat = np.concatenate([N[hs + i] for i in range(HL)], axis=1)
        xb = x[b]
        # [128, 2, X] k-tile-pair layout: element (i, j, c) = src[128*j + i, c]
        xtp = np.ascontiguousarray(xb.T.reshape(2, 128, L).transpose(1, 0, 2))
        mbp = np.ascontiguousarray(
            mcat.reshape(2, 128, HL * E).transpose(1, 0, 2))
        n8 = np.ascontiguousarray(
            ncat.reshape(2, 128, HL * E).transpose(1, 0, 2)).astype(bf16)
        maps.append({
            "x": xb.astype(bf16),
            "xt8": xtp.astype(f8),
            "xtb": xtp.astype(bf16),
            "mb": mbp.astype(bf16),
            "n": n8,
        })
    return maps


def kernel(x, W_qkv, W_out, _trace=False):
    from concourse.bass_utils import run_bass_kernel_spmd

    nc = _get_nc()
    maps = _in_maps(x, W_qkv, W_out)
    res = run_bass_kernel_spmd(nc, maps, core_ids=list(range(2 * B)),
                               trace=_trace)
    _cache["last_result"] = res
    outs = [m["out"] for m in res.results]
    full = np.stack([outs[2 * b] + outs[2 * b + 1] for b in range(B)])
    return full.astype(np.float32)
